# revision 40
# baseline (speedup 1.0000x reference)
"""DRAW-style read attention on Trainium2 — data-parallel over batch on 8 NeuronCores.

reference math (per batch element):
    params = h @ W.T + b                         [5]
    g_x = 64.5*(p0+1)-0.5 ; g_y likewise
    sigma2 = exp(p2) ; delta = (127/31)*exp(p3) ; gamma = exp(p4)
    mu_x[n] = g_x + (n-15.5)*delta ; mu_y likewise
    FX[n,h] = exp(-(h-mu_x[n])^2/(2 sigma2)) / (Z_n + 1e-8)    (Z_n = row sum)
    FY[m,w] likewise
    patch_i = FX @ img_i @ FY.T   for img in (x_c0..2, xhat_c0..2)
    out = gamma * flatten(patches)               [6144]

device layout per core (local batch B=32):
    params via 8 accumulated K=128 matmuls; tiny transforms on DVE/ACT
    expand per-b scalars to the (b,n)-partition layout with 0/1 selection matmuls
    filterbanks built in [bn, hw] layout (free-axis normalize), gamma folded into FY,
    then PE-transposed into FXT/FYT [hw, bn] bf16 for use as matmul rhs
    main loop over b: At[w,n] = img[h,w].T @ FXT_b ; patch[n,m] = At.T @ FYT_b
    (FX normalizer folded into FXT, FY normalizer*gamma folded into FYT, so patch
    in PSUM is final) -> ACT copy -> DMA to out rows
"""

import math

import numpy as np

import concourse.bass as bass  # noqa: F401  (import keeps bass registered)
import concourse.mybir as mybir
import concourse.tile as tile
from concourse import bacc
from concourse.bass_utils import run_bass_kernel_spmd
from concourse.masks import make_identity

F32 = mybir.dt.float32
BF16 = mybir.dt.bfloat16

NCORES = 8
B = 32          # per-core batch shard
C = 3
IMG = 128
N = 32
DH = 1024
U = 2 * C       # images per batch element: x channels 0..2 then x_hat channels 0..2
NT = (B * N) // 128   # tiles over the flattened (b, n) axis
KC = DH // 128        # contraction chunks for the params matmul
DELTA_NORM = (IMG - 1.0) / (N - 1.0)
EPS = 1e-8


def build_nc(finalize=True):
    nc = bacc.Bacc("TRN2", target_bir_lowering=False, debug=False, num_devices=NCORES)
    AFT = mybir.ActivationFunctionType
    ALU = mybir.AluOpType

    x_d = nc.declare_dram_parameter("x", [B, C, IMG, IMG], F32, isOutput=False)
    xh_d = nc.declare_dram_parameter("xh", [B, C, IMG, IMG], F32, isOutput=False)
    # h and W arrive as bf16 hi/lo splits: v = hi + lo with |lo| ~ 2^-9 |v|,
    # so hi@Whi + hi@Wlo + lo@Whi reproduces the f32 product to ~1e-5 rel
    # at bf16 matmul speed
    hTh_d = nc.declare_dram_parameter("hTh", [DH, B], BF16, isOutput=False)
    hTl_d = nc.declare_dram_parameter("hTl", [DH, B], BF16, isOutput=False)
    wTh_d = nc.declare_dram_parameter("wTh", [DH, 5], BF16, isOutput=False)
    wTl_d = nc.declare_dram_parameter("wTl", [DH, 5], BF16, isOutput=False)
    bias_d = nc.declare_dram_parameter("bias", [B, 5], F32, isOutput=False)
    E_d = nc.declare_dram_parameter("E", [NT, B, 128], BF16, isOutput=False)
    offs_d = nc.declare_dram_parameter("offs", [128, 1], F32, isOutput=False)
    grid_d = nc.declare_dram_parameter("grid", [128, IMG], F32, isOutput=False)
    colsc_d = nc.declare_dram_parameter("colsc", [B, 6], F32, isOutput=False)
    out_d = nc.declare_dram_parameter("out", [B, U * N * N], F32, isOutput=True)

    with tile.TileContext(nc) as tc:
        with (
            tc.tile_pool(name="consts", bufs=1) as consts,
            tc.tile_pool(name="fb", bufs=4) as fb,
            tc.tile_pool(name="imgf_p", bufs=4) as imgf_p,
            tc.tile_pool(name="imgb_p", bufs=3) as imgb_p,
            tc.tile_pool(name="atb_p", bufs=3) as atb_p,
            tc.tile_pool(name="outs_p", bufs=3) as outs_p,
            tc.tile_pool(name="ps_pro", bufs=1, space="PSUM") as ps_pro,
            tc.tile_pool(name="ps_tr", bufs=2, space="PSUM") as ps_tr,
            tc.tile_pool(name="ps_at", bufs=2, space="PSUM") as ps_at,
            tc.tile_pool(name="ps_pt", bufs=2, space="PSUM") as ps_pt,
        ):
            # ---- constants / small inputs ----
            hTh_sb = consts.tile([128, KC, B], BF16)
            nc.sync.dma_start(out=hTh_sb, in_=hTh_d[:].rearrange("(k p) b -> p k b", p=128))
            hTl_sb = consts.tile([128, KC, B], BF16)
            nc.sync.dma_start(out=hTl_sb, in_=hTl_d[:].rearrange("(k p) b -> p k b", p=128))
            wTh_sb = consts.tile([128, KC, 5], BF16)
            nc.sync.dma_start(out=wTh_sb, in_=wTh_d[:].rearrange("(k p) j -> p k j", p=128))
            wTl_sb = consts.tile([128, KC, 5], BF16)
            nc.sync.dma_start(out=wTl_sb, in_=wTl_d[:].rearrange("(k p) j -> p k j", p=128))
            bias_sb = consts.tile([B, 5], F32)
            nc.sync.dma_start(out=bias_sb, in_=bias_d[:])
            colsc_sb = consts.tile([B, 6], F32)
            nc.sync.dma_start(out=colsc_sb, in_=colsc_d[:])
            E_sb = consts.tile([B, NT, 128], BF16)
            nc.sync.dma_start(out=E_sb, in_=E_d[:].rearrange("t b p -> b t p"))
            offs_sb = consts.tile([128, 1], F32)
            nc.sync.dma_start(out=offs_sb, in_=offs_d[:])
            grid_sb = consts.tile([128, IMG], F32)
            nc.sync.dma_start(out=grid_sb, in_=grid_d[:])
            ident = consts.tile([128, 128], BF16)
            make_identity(nc, ident)
            zeros = consts.tile([128, 1], F32)
            nc.vector.memset(zeros, 0.0)
            # prime the ACT function table at t~0 so the 1.3us table load
            # doesn't sit on the params->filterbank critical path
            prime_t = consts.tile([1, 1], F32)
            nc.scalar.activation(prime_t, zeros[:1], AFT.Exp, scale=-1.0,
                                 bias=zeros[:1])

            # ---- params = h @ W.T + b  -> [B, 5] ----
            ps_par = ps_pro.tile([B, 5], F32)
            terms = [(hTh_sb, wTh_sb), (hTh_sb, wTl_sb), (hTl_sb, wTh_sb)]
            for k in range(KC):
                for ti, (hs, ws) in enumerate(terms):
                    nc.tensor.matmul(ps_par, hs[:, k, :], ws[:, k, :],
                                     start=(k == 0 and ti == 0),
                                     stop=(k == KC - 1 and ti == len(terms) - 1))
            tp = consts.tile([B, 5], F32)
            nc.vector.tensor_add(tp, ps_par, bias_sb)

            # ---- transforms -> tp2 cols = [g_x, g_y, s=sqrt(1/(2*sigma2)), delta, gamma]
            # cols 2..4 share one exp: exp([-0.5*p2, p3, p4]) * [sqrt(.5), 127/31, 1]
            tp2 = consts.tile([B, 5], F32)
            half = (IMG + 1) / 2.0
            nc.vector.tensor_scalar(tp2[:, 0:2], tp[:, 0:2], half, half - 0.5,
                                    op0=ALU.mult, op1=ALU.add)
            t3 = consts.tile([B, 3], F32)
            nc.vector.tensor_mul(t3, tp[:, 2:5], colsc_sb[:, 0:3])
            e3 = consts.tile([B, 3], F32)
            nc.scalar.activation(e3, t3, AFT.Exp, bias=zeros[:B])
            nc.vector.tensor_mul(tp2[:, 2:5], e3, colsc_sb[:, 3:6])

            # device-side hi/lo split of tp2 so the expansion matmuls run bf16
            # exactly (E is 0/1): expanded value = tp2h + tp2l = tp2
            tp2h = consts.tile([B, 5], BF16)
            nc.vector.tensor_copy(tp2h, tp2)
            tp2hf = consts.tile([B, 5], F32)
            nc.vector.tensor_copy(tp2hf, tp2h)
            tp2l = consts.tile([B, 5], BF16)
            nc.vector.tensor_sub(tp2l, tp2, tp2hf)

            # ---- expand per-b scalars to (b,n) partitions: ep [128, NT, 5] ----
            ps_e = ps_pro.tile([128, NT, 5], F32)
            for t in range(NT):
                nc.tensor.matmul(ps_e[:, t, :], E_sb[:, t, :], tp2h,
                                 start=True, stop=False)
                nc.tensor.matmul(ps_e[:, t, :], E_sb[:, t, :], tp2l,
                                 start=False, stop=True)
            # transposing copy so each parameter plane ep[:, j, :] is contiguous
            ep = consts.tile([128, 5, NT], F32)
            nc.vector.tensor_copy(ep.rearrange("p j t -> p t j"), ps_e)

            mu_x = consts.tile([128, NT], F32)
            nc.vector.scalar_tensor_tensor(mu_x, ep[:, 3, :], offs_sb, ep[:, 0, :],
                                           op0=ALU.mult, op1=ALU.add)
            mu_y = consts.tile([128, NT], F32)
            nc.vector.scalar_tensor_tensor(mu_y, ep[:, 3, :], offs_sb, ep[:, 1, :],
                                           op0=ALU.mult, op1=ALU.add)
            # bias terms for the Square trick: -mu*s
            nsmu_x = consts.tile([128, NT], F32)
            nc.vector.scalar_tensor_tensor(nsmu_x, mu_x, -1.0, ep[:, 2, :],
                                           op0=ALU.mult, op1=ALU.mult)
            nsmu_y = consts.tile([128, NT], F32)
            nc.vector.scalar_tensor_tensor(nsmu_y, mu_y, -1.0, ep[:, 2, :],
                                           op0=ALU.mult, op1=ALU.mult)

            # both filterbanks bf16 (matmuls run bf16); gamma folded into FY
            FXT = consts.tile([128, B * N], BF16)
            FYT = consts.tile([128, B * N], BF16)

            def fbank2(t):
                # sq = (s*grid - s*mu)^2 = (grid-mu)^2/(2 sigma2), X and Y
                # halves share one exp / reduce / reciprocal pass
                sq = fb.tile([128, 2, IMG], F32, tag="sq")
                nc.scalar.activation(sq[:, 0, :], grid_sb, AFT.Square,
                                     scale=ep[:, 2, t:t + 1], bias=nsmu_x[:, t:t + 1])
                nc.scalar.activation(sq[:, 1, :], grid_sb, AFT.Square,
                                     scale=ep[:, 2, t:t + 1], bias=nsmu_y[:, t:t + 1])
                e_un = fb.tile([128, 2, IMG], F32, tag="e_un")
                nc.scalar.activation(e_un, sq, AFT.Exp, scale=-1.0, bias=zeros)
                Z2 = fb.tile([128, 2], F32, tag="Z2")
                nc.vector.tensor_reduce(Z2, e_un, axis=mybir.AxisListType.X,
                                        op=ALU.add)
                nc.vector.tensor_scalar_add(Z2, Z2, EPS)
                invZ2 = fb.tile([128, 2], F32, tag="invZ2")
                nc.vector.reciprocal(invZ2, Z2)
                nc.vector.tensor_mul(invZ2[:, 1:2], invZ2[:, 1:2], ep[:, 4, t:t + 1])
                for j, FT in ((0, FXT), (1, FYT)):
                    Fn = fb.tile([128, IMG], BF16, tag="Fn")
                    nc.vector.tensor_scalar_mul(Fn, e_un[:, j, :], invZ2[:, j:j + 1])
                    ps_t = ps_tr.tile([128, 128], BF16, tag="ps_t")
                    nc.tensor.transpose(ps_t, Fn, ident)
                    nc.vector.tensor_copy(FT[:, t * 128:(t + 1) * 128], ps_t)

            # ---- main loop: pairs of batch elements, interleaved with the
            # filterbank tiles they depend on; mm2 pipelined one pair behind ----
            # mm2 is column-tiled: unit u lands on PSUM partitions 32*(u%4) at
            # free slot u//4, so the epilogue copy runs at full 128-partition
            # width; the output view flattens (u%4, n) back into DRAM columns
            out_v = (out_d[:]
                     .rearrange("(P b2) (i c n m) -> P (b2 i c) n m",
                                b2=2, i=2, c=C, n=N)
                     .rearrange("P (s j) n m -> P j n s m", s=3))

            def mm1(P, imgb, pp):
                ps_a = ps_at.tile([128, 2, U, N], F32)
                for b2 in range(2):
                    b = 2 * P + b2
                    for i in range(2):
                        for c in range(C):
                            nc.tensor.matmul(ps_a[:, b2, i * C + c, :],
                                             imgb[:, i, 2 * pp + b2, c, :],
                                             FXT[:, b * N:(b + 1) * N],
                                             start=True, stop=True)
                atb = atb_p.tile([128, 2, U, N], BF16, tag="atb")
                nc.vector.tensor_copy(atb, ps_a)
                return atb

            def mm2_store(P, atb):
                ps_p = ps_pt.tile([128, 3, N], F32)
                for b2 in range(2):
                    b = 2 * P + b2
                    for u in range(U):
                        up = b2 * U + u
                        j, slot = up % 4, up // 4
                        nc.tensor.matmul(ps_p[32 * j:32 * (j + 1), slot, :],
                                         atb[:, b2, u, :],
                                         FYT[:, b * N:(b + 1) * N],
                                         start=True, stop=True,
                                         tile_position=(0, 32 * j))
                outs = outs_p.tile([128, 3, N], F32, tag="outs")
                nc.vector.tensor_copy(outs, ps_p)
                # out DMA rides the gpsimd SWDGE queue (own queue, idle engine)
                nc.gpsimd.dma_start(out=out_v[P], in_=outs)

            prev = None
            fbank2(0)
            for t in range(NT):
                # quad image load (4 batch elements = this t-tile's span),
                # x on the SP HWDGE queue, x_hat on the ACT HWDGE queue
                imgf = imgf_p.tile([128, 2, 4, C, IMG], F32, tag="imgf")
                nc.sync.dma_start(out=imgf[:, 0],
                                  in_=x_d[4 * t:4 * t + 4].rearrange("b c h w -> h (b c) w"))
                nc.scalar.dma_start(out=imgf[:, 1],
                                    in_=xh_d[4 * t:4 * t + 4].rearrange("b c h w -> h (b c) w"))
                imgb = imgb_p.tile([128, 2, 4, C, IMG], BF16, tag="imgb")
                # f32->bf16 casts: x on DVE, x_hat on ACT (all-DVE for t=0
                # while ACT is on the filterbank critical path)
                nc.vector.tensor_copy(imgb[:, 0], imgf[:, 0])
                if t == 0:
                    nc.vector.tensor_copy(imgb[:, 1], imgf[:, 1])
                else:
                    nc.scalar.copy(imgb[:, 1], imgf[:, 1])
                # NEXT tile's filterbank emitted here so its ACT/DVE ops sit
                # ahead of the next quad's bulk casts in engine FIFOs but
                # behind this quad's (already-needed) casts
                if t + 1 < NT:
                    fbank2(t + 1)
                for pp in range(2):
                    P = 2 * t + pp
                    atb = mm1(P, imgb, pp)
                    if prev is not None:
                        mm2_store(*prev)
                    prev = (P, atb)
            mm2_store(*prev)

    if finalize:
        nc.finalize()
    return nc


_CACHE = {}


def _get_nc():
    if "nc" not in _CACHE:
        _CACHE["nc"] = build_nc()
    return _CACHE["nc"]


def host_constants():
    import ml_dtypes
    E = np.zeros((NT, B, 128), ml_dtypes.bfloat16)
    for t in range(NT):
        for p in range(128):
            E[t, (t * 128 + p) // N, p] = 1.0
    offs = (np.arange(128) % N - (N / 2.0 - 0.5)).astype(np.float32).reshape(128, 1)
    grid = np.broadcast_to(np.arange(IMG, dtype=np.float32), (128, IMG)).copy()
    colsc = np.broadcast_to(
        np.array([-0.5, 1.0, 1.0, math.sqrt(0.5), DELTA_NORM, 1.0], np.float32),
        (B, 6)).copy()
    return E, offs, grid, colsc


def make_in_maps(x, x_hat, h_dec_prev, W_read, b_read):
    x = np.asarray(x, np.float32)
    x_hat = np.asarray(x_hat, np.float32)
    h = np.asarray(h_dec_prev, np.float32)
    E, offs, grid, colsc = host_constants()
    import ml_dtypes
    bf16 = ml_dtypes.bfloat16

    def hilo(a):
        hi = a.astype(bf16)
        lo = (a - hi.astype(np.float32)).astype(bf16)
        return np.ascontiguousarray(hi), np.ascontiguousarray(lo)

    wT = np.ascontiguousarray(np.asarray(W_read, np.float32).T)
    wTh, wTl = hilo(wT)
    bias = np.broadcast_to(np.asarray(b_read, np.float32), (B, 5)).copy()
    in_maps = []
    for i in range(NCORES):
        sl = slice(i * B, (i + 1) * B)
        hTh, hTl = hilo(np.ascontiguousarray(h[sl].T))
        in_maps.append({
            "x": np.ascontiguousarray(x[sl]),
            "xh": np.ascontiguousarray(x_hat[sl]),
            "hTh": hTh,
            "hTl": hTl,
            "wTh": wTh,
            "wTl": wTl,
            "bias": bias,
            "E": E,
            "offs": offs,
            "grid": grid,
            "colsc": colsc,
        })
    return in_maps


def _install_ntff_hook():
    """The container's antenv package lacks axon_hooks; provide it so
    run_bass_kernel_spmd(trace=True) can capture an NTFF profile."""
    import sys
    import types
    if "antenv.axon_hooks" in sys.modules:
        return
    try:
        from trn_agent_boot.trn_boot import _ntff_profile_via_ctypes
    except ImportError:
        return
    mod = types.ModuleType("antenv.axon_hooks")
    hook = [_ntff_profile_via_ctypes("/opt/axon/libaxon_pjrt.so")]
    mod.set_axon_ntff_profile_hook = lambda h: hook.__setitem__(0, h)
    mod.get_axon_ntff_profile_hook = lambda: hook[0]
    sys.modules["antenv.axon_hooks"] = mod
    try:
        import antenv
        antenv.axon_hooks = mod
    except ImportError:
        pass


def run(inputs, trace=False, **spmd_kwargs):
    """Run on the 8 NeuronCores; returns (out [256, 6144] f32, BassKernelResults)."""
    if trace:
        _install_ntff_hook()
    nc = _get_nc()
    in_maps = make_in_maps(**inputs)
    res = run_bass_kernel_spmd(nc, in_maps, core_ids=list(range(NCORES)),
                               trace=trace, **spmd_kwargs)
    out = np.concatenate([res.results[i]["out"] for i in range(NCORES)], axis=0)
    return out.astype(np.float32, copy=False), res


def kernel(x, x_hat, h_dec_prev, W_read, b_read):
    out, _ = run(dict(x=x, x_hat=x_hat, h_dec_prev=h_dec_prev,
                      W_read=W_read, b_read=b_read))
    return out


# revision 47
# speedup vs baseline: 1.0235x; 1.0235x over previous
"""DRAW-style read attention on Trainium2 — data-parallel over batch on 8 NeuronCores.

reference math (per batch element):
    params = h @ W.T + b                         [5]
    g_x = 64.5*(p0+1)-0.5 ; g_y likewise
    sigma2 = exp(p2) ; delta = (127/31)*exp(p3) ; gamma = exp(p4)
    mu_x[n] = g_x + (n-15.5)*delta ; mu_y likewise
    FX[n,h] = exp(-(h-mu_x[n])^2/(2 sigma2)) / (Z_n + 1e-8)    (Z_n = row sum)
    FY[m,w] likewise
    patch_i = FX @ img_i @ FY.T   for img in (x_c0..2, xhat_c0..2)
    out = gamma * flatten(patches)               [6144]

device layout per core (local batch B=32):
    params via 8 accumulated K=128 matmuls; tiny transforms on DVE/ACT
    expand per-b scalars to the (b,n)-partition layout with 0/1 selection matmuls
    filterbanks built in [bn, hw] layout (free-axis normalize), gamma folded into FY,
    then PE-transposed into FXT/FYT [hw, bn] bf16 for use as matmul rhs
    main loop over b: At[w,n] = img[h,w].T @ FXT_b ; patch[n,m] = At.T @ FYT_b
    (FX normalizer folded into FXT, FY normalizer*gamma folded into FYT, so patch
    in PSUM is final) -> ACT copy -> DMA to out rows
"""

import math

import numpy as np

import concourse.bass as bass  # noqa: F401  (import keeps bass registered)
import concourse.mybir as mybir
import concourse.tile as tile
from concourse import bacc
from concourse.bass_utils import run_bass_kernel_spmd
from concourse.masks import make_identity

F32 = mybir.dt.float32
BF16 = mybir.dt.bfloat16

NCORES = 8
B = 32          # per-core batch shard
C = 3
IMG = 128
N = 32
DH = 1024
U = 2 * C       # images per batch element: x channels 0..2 then x_hat channels 0..2
NT = (B * N) // 128   # tiles over the flattened (b, n) axis
KC = DH // 128        # contraction chunks for the params matmul
DELTA_NORM = (IMG - 1.0) / (N - 1.0)
EPS = 1e-8


def build_nc(finalize=True):
    nc = bacc.Bacc("TRN2", target_bir_lowering=False, debug=False, num_devices=NCORES)
    AFT = mybir.ActivationFunctionType
    ALU = mybir.AluOpType

    x_d = nc.declare_dram_parameter("x", [B, C, IMG, IMG], F32, isOutput=False)
    xh_d = nc.declare_dram_parameter("xh", [B, C, IMG, IMG], F32, isOutput=False)
    # h and W arrive as bf16 hi/lo splits packed side by side: v = hi + lo with
    # |lo| ~ 2^-9 |v|, so hi@Whi + hi@Wlo + lo@Whi reproduces the f32 product
    # to ~1e-5 rel at bf16 matmul speed. Constants are packed into few tensors
    # so the startup isn't serialized by per-DMA queue overheads.
    hT2_d = nc.declare_dram_parameter("hT2", [DH, 2 * B], BF16, isOutput=False)
    wT2_d = nc.declare_dram_parameter("wT2", [DH, 10], BF16, isOutput=False)
    bc_d = nc.declare_dram_parameter("bc", [B, 11], F32, isOutput=False)
    E_d = nc.declare_dram_parameter("E", [NT, B, 128], BF16, isOutput=False)
    go_d = nc.declare_dram_parameter("go", [128, IMG + 1], F32, isOutput=False)
    out_d = nc.declare_dram_parameter("out", [B, U * N * N], F32, isOutput=True)

    with tile.TileContext(nc) as tc:
        with (
            tc.tile_pool(name="consts", bufs=1) as consts,
            tc.tile_pool(name="fb", bufs=4) as fb,
            tc.tile_pool(name="imgf_p", bufs=4) as imgf_p,
            tc.tile_pool(name="imgb_p", bufs=3) as imgb_p,
            tc.tile_pool(name="atb_p", bufs=3) as atb_p,
            tc.tile_pool(name="outs_p", bufs=3) as outs_p,
            tc.tile_pool(name="ps_pro", bufs=1, space="PSUM") as ps_pro,
            tc.tile_pool(name="ps_tr", bufs=2, space="PSUM") as ps_tr,
            tc.tile_pool(name="ps_at", bufs=2, space="PSUM") as ps_at,
            tc.tile_pool(name="ps_pt", bufs=2, space="PSUM") as ps_pt,
        ):
            # ---- constants / small inputs ----
            # hot consts (params deps) first on the SP queue, ahead of images
            hT2_sb = consts.tile([128, KC, 2 * B], BF16)
            nc.sync.dma_start(out=hT2_sb, in_=hT2_d[:].rearrange("(k p) b -> p k b", p=128))
            wT2_sb = consts.tile([128, KC, 10], BF16)
            nc.sync.dma_start(out=wT2_sb, in_=wT2_d[:].rearrange("(k p) j -> p k j", p=128))
            bc_sb = consts.tile([B, 11], F32)
            nc.sync.dma_start(out=bc_sb, in_=bc_d[:])
            bias_sb = bc_sb[:, 0:5]
            colsc_sb = bc_sb[:, 5:11]
            # cold consts (expansion/filterbank deps) ride the ACT queue
            E_sb = consts.tile([B, NT, 128], BF16)
            nc.scalar.dma_start(out=E_sb, in_=E_d[:].rearrange("t b p -> b t p"))
            go_sb = consts.tile([128, IMG + 1], F32)
            nc.scalar.dma_start(out=go_sb, in_=go_d[:])
            grid_sb = go_sb[:, 0:IMG]
            offs_sb = go_sb[:, IMG:IMG + 1]
            ident = consts.tile([128, 128], BF16)
            make_identity(nc, ident)
            zeros = consts.tile([128, 1], F32)
            nc.vector.memset(zeros, 0.0)
            # prime the ACT function table at t~0 so the 1.3us table load
            # doesn't sit on the params->filterbank critical path
            prime_t = consts.tile([1, 1], F32)
            nc.scalar.activation(prime_t, zeros[:1], AFT.Exp, scale=-1.0,
                                 bias=zeros[:1])

            # ---- quad image loads: x on the SP HWDGE queue, x_hat on the ACT
            # HWDGE queue; f32->bf16 casts on DVE (x) and ACT (x_hat). quad 0
            # is emitted before the params chain so its data is on-chip by the
            # time FXT(0) is ready.
            def emit_quad(t):
                imgf = imgf_p.tile([128, 2, 4, C, IMG], F32, tag="imgf",
                                   name=f"imgf{t}")
                nc.sync.dma_start(out=imgf[:, 0],
                                  in_=x_d[4 * t:4 * t + 4].rearrange("b c h w -> h (b c) w"))
                nc.scalar.dma_start(out=imgf[:, 1],
                                    in_=xh_d[4 * t:4 * t + 4].rearrange("b c h w -> h (b c) w"))
                imgb = imgb_p.tile([128, 2, 4, C, IMG], BF16, tag="imgb",
                                   name=f"imgb{t}")
                nc.vector.tensor_copy(imgb[:, 0], imgf[:, 0])
                nc.scalar.copy(imgb[:, 1], imgf[:, 1])
                return imgb

            quad_imgb = {0: emit_quad(0)}

            # ---- params = h @ W.T + b  -> [B, 5] ----
            ps_par = ps_pro.tile([B, 5], F32)
            terms = [(slice(0, B), slice(0, 5)), (slice(0, B), slice(5, 10)),
                     (slice(B, 2 * B), slice(0, 5))]
            for k in range(KC):
                for ti, (hsl, wsl) in enumerate(terms):
                    nc.tensor.matmul(ps_par, hT2_sb[:, k, hsl], wT2_sb[:, k, wsl],
                                     start=(k == 0 and ti == 0),
                                     stop=(k == KC - 1 and ti == len(terms) - 1))
            tp = consts.tile([B, 5], F32)
            nc.vector.tensor_add(tp, ps_par, bias_sb)

            # ---- transforms -> tp2 cols = [g_x, g_y, s=sqrt(1/(2*sigma2)), delta, gamma]
            # cols 2..4 share one exp: exp([-0.5*p2, p3, p4]) * [sqrt(.5), 127/31, 1]
            tp2 = consts.tile([B, 5], F32)
            half = (IMG + 1) / 2.0
            nc.vector.tensor_scalar(tp2[:, 0:2], tp[:, 0:2], half, half - 0.5,
                                    op0=ALU.mult, op1=ALU.add)
            t3 = consts.tile([B, 3], F32)
            nc.vector.tensor_mul(t3, tp[:, 2:5], colsc_sb[:, 0:3])
            e3 = consts.tile([B, 3], F32)
            nc.scalar.activation(e3, t3, AFT.Exp, bias=zeros[:B])
            nc.vector.tensor_mul(tp2[:, 2:5], e3, colsc_sb[:, 3:6])

            # device-side hi/lo split of tp2 so the expansion matmuls run bf16
            # exactly (E is 0/1): expanded value = tp2h + tp2l = tp2
            tp2h = consts.tile([B, 5], BF16)
            nc.vector.tensor_copy(tp2h, tp2)
            tp2hf = consts.tile([B, 5], F32)
            nc.vector.tensor_copy(tp2hf, tp2h)
            tp2l = consts.tile([B, 5], BF16)
            nc.vector.tensor_sub(tp2l, tp2, tp2hf)

            # ---- expand per-b scalars to (b,n) partitions: ep [128, NT, 5] ----
            ps_e = ps_pro.tile([128, NT, 5], F32)
            for t in range(NT):
                nc.tensor.matmul(ps_e[:, t, :], E_sb[:, t, :], tp2h,
                                 start=True, stop=False)
                nc.tensor.matmul(ps_e[:, t, :], E_sb[:, t, :], tp2l,
                                 start=False, stop=True)
            # transposing copy so each parameter plane ep[:, j, :] is contiguous
            ep = consts.tile([128, 5, NT], F32)
            nc.vector.tensor_copy(ep.rearrange("p j t -> p t j"), ps_e)

            mu_x = consts.tile([128, NT], F32)
            nc.vector.scalar_tensor_tensor(mu_x, ep[:, 3, :], offs_sb, ep[:, 0, :],
                                           op0=ALU.mult, op1=ALU.add)
            mu_y = consts.tile([128, NT], F32)
            nc.vector.scalar_tensor_tensor(mu_y, ep[:, 3, :], offs_sb, ep[:, 1, :],
                                           op0=ALU.mult, op1=ALU.add)
            # bias terms for the Square trick: -mu*s
            nsmu_x = consts.tile([128, NT], F32)
            nc.vector.scalar_tensor_tensor(nsmu_x, mu_x, -1.0, ep[:, 2, :],
                                           op0=ALU.mult, op1=ALU.mult)
            nsmu_y = consts.tile([128, NT], F32)
            nc.vector.scalar_tensor_tensor(nsmu_y, mu_y, -1.0, ep[:, 2, :],
                                           op0=ALU.mult, op1=ALU.mult)

            # both filterbanks bf16 (matmuls run bf16); gamma folded into FY
            FXT = consts.tile([128, B * N], BF16)
            FYT = consts.tile([128, B * N], BF16)

            def fbank2(t):
                # sq = (s*grid - s*mu)^2 = (grid-mu)^2/(2 sigma2), X and Y
                # halves share one exp / reduce / reciprocal pass
                sq = fb.tile([128, 2, IMG], F32, tag="sq")
                nc.scalar.activation(sq[:, 0, :], grid_sb, AFT.Square,
                                     scale=ep[:, 2, t:t + 1], bias=nsmu_x[:, t:t + 1])
                nc.scalar.activation(sq[:, 1, :], grid_sb, AFT.Square,
                                     scale=ep[:, 2, t:t + 1], bias=nsmu_y[:, t:t + 1])
                e_un = fb.tile([128, 2, IMG], F32, tag="e_un")
                nc.scalar.activation(e_un, sq, AFT.Exp, scale=-1.0, bias=zeros)
                Z2 = fb.tile([128, 2], F32, tag="Z2")
                nc.vector.tensor_reduce(Z2, e_un, axis=mybir.AxisListType.X,
                                        op=ALU.add)
                nc.vector.tensor_scalar_add(Z2, Z2, EPS)
                invZ2 = fb.tile([128, 2], F32, tag="invZ2")
                nc.vector.reciprocal(invZ2, Z2)
                nc.vector.tensor_mul(invZ2[:, 1:2], invZ2[:, 1:2], ep[:, 4, t:t + 1])
                for j, FT in ((0, FXT), (1, FYT)):
                    Fn = fb.tile([128, IMG], BF16, tag="Fn")
                    nc.vector.tensor_scalar_mul(Fn, e_un[:, j, :], invZ2[:, j:j + 1])
                    ps_t = ps_tr.tile([128, 128], BF16, tag="ps_t")
                    nc.tensor.transpose(ps_t, Fn, ident)
                    nc.vector.tensor_copy(FT[:, t * 128:(t + 1) * 128], ps_t)

            # ---- main loop: pairs of batch elements, interleaved with the
            # filterbank tiles they depend on; mm2 pipelined one pair behind ----
            # mm2 is column-tiled: unit u lands on PSUM partitions 32*(u%4) at
            # free slot u//4, so the epilogue copy runs at full 128-partition
            # width; the output view flattens (u%4, n) back into DRAM columns
            out_v = (out_d[:]
                     .rearrange("(P b2) (i c n m) -> P (b2 i c) n m",
                                b2=2, i=2, c=C, n=N)
                     .rearrange("P (s j) n m -> P j n s m", s=3))

            def mm1(P, imgb, pp):
                ps_a = ps_at.tile([128, 2, U, N], F32)
                for b2 in range(2):
                    b = 2 * P + b2
                    for i in range(2):
                        for c in range(C):
                            nc.tensor.matmul(ps_a[:, b2, i * C + c, :],
                                             imgb[:, i, 2 * pp + b2, c, :],
                                             FXT[:, b * N:(b + 1) * N],
                                             start=True, stop=True)
                atb = atb_p.tile([128, 2, U, N], BF16, tag="atb")
                nc.vector.tensor_copy(atb, ps_a)
                return atb

            def mm2_store(P, atb):
                ps_p = ps_pt.tile([128, 3, N], F32)
                for b2 in range(2):
                    b = 2 * P + b2
                    for u in range(U):
                        up = b2 * U + u
                        j, slot = up % 4, up // 4
                        nc.tensor.matmul(ps_p[32 * j:32 * (j + 1), slot, :],
                                         atb[:, b2, u, :],
                                         FYT[:, b * N:(b + 1) * N],
                                         start=True, stop=True,
                                         tile_position=(0, 32 * j))
                outs = outs_p.tile([128, 3, N], F32, tag="outs")
                nc.vector.tensor_copy(outs, ps_p)
                # out DMA rides the gpsimd SWDGE queue (own queue, idle engine)
                nc.gpsimd.dma_start(out=out_v[P], in_=outs)

            prev = None
            fbank2(0)
            fbank2(1)
            for t in range(NT):
                # quads stay one tile ahead, filterbanks two ahead, so each
                # dependency is ready before its consumers reach the engines
                if t + 1 < NT:
                    quad_imgb[t + 1] = emit_quad(t + 1)
                if t + 2 < NT:
                    fbank2(t + 2)
                imgb = quad_imgb.pop(t)
                for pp in range(2):
                    P = 2 * t + pp
                    atb = mm1(P, imgb, pp)
                    if prev is not None:
                        mm2_store(*prev)
                    prev = (P, atb)
            mm2_store(*prev)

    if finalize:
        nc.finalize()
    return nc


_CACHE = {}


def _get_nc():
    if "nc" not in _CACHE:
        _CACHE["nc"] = build_nc()
    return _CACHE["nc"]


def host_constants():
    import ml_dtypes
    E = np.zeros((NT, B, 128), ml_dtypes.bfloat16)
    for t in range(NT):
        for p in range(128):
            E[t, (t * 128 + p) // N, p] = 1.0
    offs = (np.arange(128) % N - (N / 2.0 - 0.5)).astype(np.float32).reshape(128, 1)
    grid = np.broadcast_to(np.arange(IMG, dtype=np.float32), (128, IMG))
    go = np.ascontiguousarray(np.concatenate([grid, offs], axis=1))
    colsc = np.broadcast_to(
        np.array([-0.5, 1.0, 1.0, math.sqrt(0.5), DELTA_NORM, 1.0], np.float32),
        (B, 6))
    return E, go, colsc


def make_in_maps(x, x_hat, h_dec_prev, W_read, b_read):
    x = np.asarray(x, np.float32)
    x_hat = np.asarray(x_hat, np.float32)
    h = np.asarray(h_dec_prev, np.float32)
    E, go, colsc = host_constants()
    import ml_dtypes
    bf16 = ml_dtypes.bfloat16

    def hilo2(a):
        hi = a.astype(bf16)
        lo = (a - hi.astype(np.float32)).astype(bf16)
        return np.ascontiguousarray(np.concatenate([hi, lo], axis=1))

    wT2 = hilo2(np.asarray(W_read, np.float32).T)
    bias = np.broadcast_to(np.asarray(b_read, np.float32), (B, 5))
    bc = np.ascontiguousarray(np.concatenate([bias, colsc], axis=1))
    in_maps = []
    for i in range(NCORES):
        sl = slice(i * B, (i + 1) * B)
        in_maps.append({
            "x": np.ascontiguousarray(x[sl]),
            "xh": np.ascontiguousarray(x_hat[sl]),
            "hT2": hilo2(np.ascontiguousarray(h[sl].T)),
            "wT2": wT2,
            "bc": bc,
            "E": E,
            "go": go,
        })
    return in_maps


def _install_ntff_hook():
    """The container's antenv package lacks axon_hooks; provide it so
    run_bass_kernel_spmd(trace=True) can capture an NTFF profile."""
    import sys
    import types
    if "antenv.axon_hooks" in sys.modules:
        return
    try:
        from trn_agent_boot.trn_boot import _ntff_profile_via_ctypes
    except ImportError:
        return
    mod = types.ModuleType("antenv.axon_hooks")
    hook = [_ntff_profile_via_ctypes("/opt/axon/libaxon_pjrt.so")]
    mod.set_axon_ntff_profile_hook = lambda h: hook.__setitem__(0, h)
    mod.get_axon_ntff_profile_hook = lambda: hook[0]
    sys.modules["antenv.axon_hooks"] = mod
    try:
        import antenv
        antenv.axon_hooks = mod
    except ImportError:
        pass


def run(inputs, trace=False, **spmd_kwargs):
    """Run on the 8 NeuronCores; returns (out [256, 6144] f32, BassKernelResults)."""
    if trace:
        _install_ntff_hook()
    nc = _get_nc()
    in_maps = make_in_maps(**inputs)
    res = run_bass_kernel_spmd(nc, in_maps, core_ids=list(range(NCORES)),
                               trace=trace, **spmd_kwargs)
    out = np.concatenate([res.results[i]["out"] for i in range(NCORES)], axis=0)
    return out.astype(np.float32, copy=False), res


def kernel(x, x_hat, h_dec_prev, W_read, b_read):
    out, _ = run(dict(x=x, x_hat=x_hat, h_dec_prev=h_dec_prev,
                      W_read=W_read, b_read=b_read))
    return out


# revision 49
# speedup vs baseline: 1.0583x; 1.0340x over previous
"""DRAW-style read attention on Trainium2 — data-parallel over batch on 8 NeuronCores.

reference math (per batch element):
    params = h @ W.T + b                         [5]
    g_x = 64.5*(p0+1)-0.5 ; g_y likewise
    sigma2 = exp(p2) ; delta = (127/31)*exp(p3) ; gamma = exp(p4)
    mu_x[n] = g_x + (n-15.5)*delta ; mu_y likewise
    FX[n,h] = exp(-(h-mu_x[n])^2/(2 sigma2)) / (Z_n + 1e-8)    (Z_n = row sum)
    FY[m,w] likewise
    patch_i = FX @ img_i @ FY.T   for img in (x_c0..2, xhat_c0..2)
    out = gamma * flatten(patches)               [6144]

device layout per core (local batch B=32):
    params via 8 accumulated K=128 matmuls; tiny transforms on DVE/ACT
    expand per-b scalars to the (b,n)-partition layout with 0/1 selection matmuls
    filterbanks built in [bn, hw] layout (free-axis normalize), gamma folded into FY,
    then PE-transposed into FXT/FYT [hw, bn] bf16 for use as matmul rhs
    main loop over b: At[w,n] = img[h,w].T @ FXT_b ; patch[n,m] = At.T @ FYT_b
    (FX normalizer folded into FXT, FY normalizer*gamma folded into FYT, so patch
    in PSUM is final) -> ACT copy -> DMA to out rows
"""

import math

import numpy as np

import concourse.bass as bass  # noqa: F401  (import keeps bass registered)
import concourse.mybir as mybir
import concourse.tile as tile
from concourse import bacc
from concourse.bass_utils import run_bass_kernel_spmd
from concourse.masks import make_identity

F32 = mybir.dt.float32
BF16 = mybir.dt.bfloat16

NCORES = 8
B = 32          # per-core batch shard
C = 3
IMG = 128
N = 32
DH = 1024
U = 2 * C       # images per batch element: x channels 0..2 then x_hat channels 0..2
NT = (B * N) // 128   # tiles over the flattened (b, n) axis
KC = DH // 128        # contraction chunks for the params matmul
DELTA_NORM = (IMG - 1.0) / (N - 1.0)
EPS = 1e-8


def build_nc(finalize=True):
    nc = bacc.Bacc("TRN2", target_bir_lowering=False, debug=False, num_devices=NCORES)
    AFT = mybir.ActivationFunctionType
    ALU = mybir.AluOpType

    x_d = nc.declare_dram_parameter("x", [B, C, IMG, IMG], F32, isOutput=False)
    xh_d = nc.declare_dram_parameter("xh", [B, C, IMG, IMG], F32, isOutput=False)
    # h and W arrive as bf16 hi/lo splits packed side by side: v = hi + lo with
    # |lo| ~ 2^-9 |v|, so hi@Whi + hi@Wlo + lo@Whi reproduces the f32 product
    # to ~1e-5 rel at bf16 matmul speed. Constants are packed into few tensors
    # so the startup isn't serialized by per-DMA queue overheads.
    hT2_d = nc.declare_dram_parameter("hT2", [DH, 2 * B], BF16, isOutput=False)
    wT2_d = nc.declare_dram_parameter("wT2", [DH, 10], BF16, isOutput=False)
    bc_d = nc.declare_dram_parameter("bc", [B, 11], F32, isOutput=False)
    E_d = nc.declare_dram_parameter("E", [NT, B, 128], BF16, isOutput=False)
    go_d = nc.declare_dram_parameter("go", [128, IMG + 1], F32, isOutput=False)
    out_d = nc.declare_dram_parameter("out", [B, U * N * N], F32, isOutput=True)

    with tile.TileContext(nc) as tc:
        with (
            tc.tile_pool(name="consts", bufs=1) as consts,
            tc.tile_pool(name="fb", bufs=4) as fb,
            tc.tile_pool(name="imgf_p", bufs=4) as imgf_p,
            tc.tile_pool(name="imgb_p", bufs=3) as imgb_p,
            tc.tile_pool(name="atb_p", bufs=3) as atb_p,
            tc.tile_pool(name="outs_p", bufs=3) as outs_p,
            tc.tile_pool(name="ps_pro", bufs=1, space="PSUM") as ps_pro,
            tc.tile_pool(name="ps_tr", bufs=2, space="PSUM") as ps_tr,
            tc.tile_pool(name="ps_at", bufs=2, space="PSUM") as ps_at,
            tc.tile_pool(name="ps_pt", bufs=2, space="PSUM") as ps_pt,
        ):
            # ---- constants / small inputs ----
            # hot consts (params deps) first on the SP queue, ahead of images
            hT2_sb = consts.tile([128, KC, 2 * B], BF16)
            nc.sync.dma_start(out=hT2_sb, in_=hT2_d[:].rearrange("(k p) b -> p k b", p=128))
            wT2_sb = consts.tile([128, KC, 10], BF16)
            nc.sync.dma_start(out=wT2_sb, in_=wT2_d[:].rearrange("(k p) j -> p k j", p=128))
            bc_sb = consts.tile([B, 11], F32)
            nc.sync.dma_start(out=bc_sb, in_=bc_d[:])
            bias_sb = bc_sb[:, 0:5]
            colsc_sb = bc_sb[:, 5:11]
            # cold consts (expansion/filterbank deps) ride the ACT queue
            E_sb = consts.tile([B, NT, 128], BF16)
            nc.scalar.dma_start(out=E_sb, in_=E_d[:].rearrange("t b p -> b t p"))
            go_sb = consts.tile([128, IMG + 1], F32)
            nc.scalar.dma_start(out=go_sb, in_=go_d[:])
            grid_sb = go_sb[:, 0:IMG]
            offs_sb = go_sb[:, IMG:IMG + 1]
            ident = consts.tile([128, 128], BF16)
            make_identity(nc, ident)
            zeros = consts.tile([128, 1], F32)
            nc.vector.memset(zeros, 0.0)
            # prime the ACT function table at t~0 so the 1.3us table load
            # doesn't sit on the params->filterbank critical path
            prime_t = consts.tile([1, 1], F32)
            nc.scalar.activation(prime_t, zeros[:1], AFT.Exp, scale=-1.0,
                                 bias=zeros[:1])

            # ---- quad image loads: x on the SP HWDGE queue, x_hat on the ACT
            # HWDGE queue; f32->bf16 casts on DVE (x) and ACT (x_hat). DMAs
            # are issued as early as possible; the casts are emitted late so
            # the static per-engine schedule never parks a DMA-gated cast in
            # front of critical-path compute.
            def emit_quad_dma(t):
                imgf = imgf_p.tile([128, 2, 4, C, IMG], F32, tag="imgf",
                                   name=f"imgf{t}")
                nc.sync.dma_start(out=imgf[:, 0],
                                  in_=x_d[4 * t:4 * t + 4].rearrange("b c h w -> h (b c) w"))
                nc.scalar.dma_start(out=imgf[:, 1],
                                    in_=xh_d[4 * t:4 * t + 4].rearrange("b c h w -> h (b c) w"))
                return imgf

            def emit_quad_cast(t, imgf):
                imgb = imgb_p.tile([128, 2, 4, C, IMG], BF16, tag="imgb",
                                   name=f"imgb{t}")
                nc.vector.tensor_copy(imgb[:, 0], imgf[:, 0])
                nc.scalar.copy(imgb[:, 1], imgf[:, 1])
                return imgb

            quad_imgf = {0: emit_quad_dma(0), 1: emit_quad_dma(1)}
            quad_imgb = {}

            # ---- params = h @ W.T + b  -> [B, 5] ----
            ps_par = ps_pro.tile([B, 5], F32)
            terms = [(slice(0, B), slice(0, 5)), (slice(0, B), slice(5, 10)),
                     (slice(B, 2 * B), slice(0, 5))]
            for k in range(KC):
                for ti, (hsl, wsl) in enumerate(terms):
                    nc.tensor.matmul(ps_par, hT2_sb[:, k, hsl], wT2_sb[:, k, wsl],
                                     start=(k == 0 and ti == 0),
                                     stop=(k == KC - 1 and ti == len(terms) - 1))
            tp = consts.tile([B, 5], F32)
            nc.vector.tensor_add(tp, ps_par, bias_sb)

            # ---- transforms -> tp2 cols = [g_x, g_y, s=sqrt(1/(2*sigma2)), delta, gamma]
            # cols 2..4 share one exp: exp([-0.5*p2, p3, p4]) * [sqrt(.5), 127/31, 1]
            tp2 = consts.tile([B, 5], F32)
            half = (IMG + 1) / 2.0
            nc.vector.tensor_scalar(tp2[:, 0:2], tp[:, 0:2], half, half - 0.5,
                                    op0=ALU.mult, op1=ALU.add)
            t3 = consts.tile([B, 3], F32)
            nc.vector.tensor_mul(t3, tp[:, 2:5], colsc_sb[:, 0:3])
            e3 = consts.tile([B, 3], F32)
            nc.scalar.activation(e3, t3, AFT.Exp, bias=zeros[:B])
            nc.vector.tensor_mul(tp2[:, 2:5], e3, colsc_sb[:, 3:6])

            # device-side hi/lo split of tp2 so the expansion matmuls run bf16
            # exactly (E is 0/1): expanded value = tp2h + tp2l = tp2
            tp2h = consts.tile([B, 5], BF16)
            nc.vector.tensor_copy(tp2h, tp2)
            tp2hf = consts.tile([B, 5], F32)
            nc.vector.tensor_copy(tp2hf, tp2h)
            tp2l = consts.tile([B, 5], BF16)
            nc.vector.tensor_sub(tp2l, tp2, tp2hf)

            # ---- expand per-b scalars to (b,n) partitions: ep [128, NT, 5] ----
            ps_e = ps_pro.tile([128, NT, 5], F32)
            for t in range(NT):
                nc.tensor.matmul(ps_e[:, t, :], E_sb[:, t, :], tp2h,
                                 start=True, stop=False)
                nc.tensor.matmul(ps_e[:, t, :], E_sb[:, t, :], tp2l,
                                 start=False, stop=True)
            # transposing copy so each parameter plane ep[:, j, :] is contiguous
            ep = consts.tile([128, 5, NT], F32)
            nc.vector.tensor_copy(ep.rearrange("p j t -> p t j"), ps_e)

            mu_x = consts.tile([128, NT], F32)
            nc.vector.scalar_tensor_tensor(mu_x, ep[:, 3, :], offs_sb, ep[:, 0, :],
                                           op0=ALU.mult, op1=ALU.add)
            mu_y = consts.tile([128, NT], F32)
            nc.vector.scalar_tensor_tensor(mu_y, ep[:, 3, :], offs_sb, ep[:, 1, :],
                                           op0=ALU.mult, op1=ALU.add)
            # bias terms for the Square trick: -mu*s
            nsmu_x = consts.tile([128, NT], F32)
            nc.vector.scalar_tensor_tensor(nsmu_x, mu_x, -1.0, ep[:, 2, :],
                                           op0=ALU.mult, op1=ALU.mult)
            nsmu_y = consts.tile([128, NT], F32)
            nc.vector.scalar_tensor_tensor(nsmu_y, mu_y, -1.0, ep[:, 2, :],
                                           op0=ALU.mult, op1=ALU.mult)

            # both filterbanks bf16 (matmuls run bf16); gamma folded into FY
            FXT = consts.tile([128, B * N], BF16)
            FYT = consts.tile([128, B * N], BF16)

            def fbank2(t):
                # sq = (s*grid - s*mu)^2 = (grid-mu)^2/(2 sigma2), X and Y
                # halves share one exp / reduce / reciprocal pass
                sq = fb.tile([128, 2, IMG], F32, tag="sq")
                nc.scalar.activation(sq[:, 0, :], grid_sb, AFT.Square,
                                     scale=ep[:, 2, t:t + 1], bias=nsmu_x[:, t:t + 1])
                nc.scalar.activation(sq[:, 1, :], grid_sb, AFT.Square,
                                     scale=ep[:, 2, t:t + 1], bias=nsmu_y[:, t:t + 1])
                e_un = fb.tile([128, 2, IMG], F32, tag="e_un")
                nc.scalar.activation(e_un, sq, AFT.Exp, scale=-1.0, bias=zeros)
                Z2 = fb.tile([128, 2], F32, tag="Z2")
                nc.vector.tensor_reduce(Z2, e_un, axis=mybir.AxisListType.X,
                                        op=ALU.add)
                nc.vector.tensor_scalar_add(Z2, Z2, EPS)
                invZ2 = fb.tile([128, 2], F32, tag="invZ2")
                nc.vector.reciprocal(invZ2, Z2)
                nc.vector.tensor_mul(invZ2[:, 1:2], invZ2[:, 1:2], ep[:, 4, t:t + 1])
                for j, FT in ((0, FXT), (1, FYT)):
                    Fn = fb.tile([128, IMG], BF16, tag="Fn")
                    nc.vector.tensor_scalar_mul(Fn, e_un[:, j, :], invZ2[:, j:j + 1])
                    ps_t = ps_tr.tile([128, 128], BF16, tag="ps_t")
                    nc.tensor.transpose(ps_t, Fn, ident)
                    nc.vector.tensor_copy(FT[:, t * 128:(t + 1) * 128], ps_t)

            # ---- main loop: pairs of batch elements, interleaved with the
            # filterbank tiles they depend on; mm2 pipelined one pair behind ----
            # mm2 is column-tiled: unit u lands on PSUM partitions 32*(u%4) at
            # free slot u//4, so the epilogue copy runs at full 128-partition
            # width; the output view flattens (u%4, n) back into DRAM columns
            out_v = (out_d[:]
                     .rearrange("(P b2) (i c n m) -> P (b2 i c) n m",
                                b2=2, i=2, c=C, n=N)
                     .rearrange("P (s j) n m -> P j n s m", s=3))

            def mm1(P, imgb, pp):
                ps_a = ps_at.tile([128, 2, U, N], F32)
                for b2 in range(2):
                    b = 2 * P + b2
                    for i in range(2):
                        for c in range(C):
                            nc.tensor.matmul(ps_a[:, b2, i * C + c, :],
                                             imgb[:, i, 2 * pp + b2, c, :],
                                             FXT[:, b * N:(b + 1) * N],
                                             start=True, stop=True)
                atb = atb_p.tile([128, 2, U, N], BF16, tag="atb")
                nc.vector.tensor_copy(atb, ps_a)
                return atb

            def mm2_store(P, atb):
                ps_p = ps_pt.tile([128, 3, N], F32)
                for b2 in range(2):
                    b = 2 * P + b2
                    for u in range(U):
                        up = b2 * U + u
                        j, slot = up % 4, up // 4
                        nc.tensor.matmul(ps_p[32 * j:32 * (j + 1), slot, :],
                                         atb[:, b2, u, :],
                                         FYT[:, b * N:(b + 1) * N],
                                         start=True, stop=True,
                                         tile_position=(0, 32 * j))
                outs = outs_p.tile([128, 3, N], F32, tag="outs")
                nc.vector.tensor_copy(outs, ps_p)
                # out DMA rides the gpsimd SWDGE queue (own queue, idle engine)
                nc.gpsimd.dma_start(out=out_v[P], in_=outs)

            prev = None
            fbank2(0)
            fbank2(1)
            quad_imgb[0] = emit_quad_cast(0, quad_imgf.pop(0))
            for t in range(NT):
                # DMAs two quads ahead, filterbanks two tiles ahead, casts one
                # quad ahead: every dependency lands before its consumer
                if t + 2 < NT:
                    quad_imgf[t + 2] = emit_quad_dma(t + 2)
                    fbank2(t + 2)
                if t + 1 < NT:
                    quad_imgb[t + 1] = emit_quad_cast(t + 1, quad_imgf.pop(t + 1))
                imgb = quad_imgb.pop(t)
                for pp in range(2):
                    P = 2 * t + pp
                    atb = mm1(P, imgb, pp)
                    if prev is not None:
                        mm2_store(*prev)
                    prev = (P, atb)
            mm2_store(*prev)

    if finalize:
        nc.finalize()
    return nc


_CACHE = {}


def _get_nc():
    if "nc" not in _CACHE:
        _CACHE["nc"] = build_nc()
    return _CACHE["nc"]


def host_constants():
    import ml_dtypes
    E = np.zeros((NT, B, 128), ml_dtypes.bfloat16)
    for t in range(NT):
        for p in range(128):
            E[t, (t * 128 + p) // N, p] = 1.0
    offs = (np.arange(128) % N - (N / 2.0 - 0.5)).astype(np.float32).reshape(128, 1)
    grid = np.broadcast_to(np.arange(IMG, dtype=np.float32), (128, IMG))
    go = np.ascontiguousarray(np.concatenate([grid, offs], axis=1))
    colsc = np.broadcast_to(
        np.array([-0.5, 1.0, 1.0, math.sqrt(0.5), DELTA_NORM, 1.0], np.float32),
        (B, 6))
    return E, go, colsc


def make_in_maps(x, x_hat, h_dec_prev, W_read, b_read):
    x = np.asarray(x, np.float32)
    x_hat = np.asarray(x_hat, np.float32)
    h = np.asarray(h_dec_prev, np.float32)
    E, go, colsc = host_constants()
    import ml_dtypes
    bf16 = ml_dtypes.bfloat16

    def hilo2(a):
        hi = a.astype(bf16)
        lo = (a - hi.astype(np.float32)).astype(bf16)
        return np.ascontiguousarray(np.concatenate([hi, lo], axis=1))

    wT2 = hilo2(np.asarray(W_read, np.float32).T)
    bias = np.broadcast_to(np.asarray(b_read, np.float32), (B, 5))
    bc = np.ascontiguousarray(np.concatenate([bias, colsc], axis=1))
    in_maps = []
    for i in range(NCORES):
        sl = slice(i * B, (i + 1) * B)
        in_maps.append({
            "x": np.ascontiguousarray(x[sl]),
            "xh": np.ascontiguousarray(x_hat[sl]),
            "hT2": hilo2(np.ascontiguousarray(h[sl].T)),
            "wT2": wT2,
            "bc": bc,
            "E": E,
            "go": go,
        })
    return in_maps


def _install_ntff_hook():
    """The container's antenv package lacks axon_hooks; provide it so
    run_bass_kernel_spmd(trace=True) can capture an NTFF profile."""
    import sys
    import types
    if "antenv.axon_hooks" in sys.modules:
        return
    try:
        from trn_agent_boot.trn_boot import _ntff_profile_via_ctypes
    except ImportError:
        return
    mod = types.ModuleType("antenv.axon_hooks")
    hook = [_ntff_profile_via_ctypes("/opt/axon/libaxon_pjrt.so")]
    mod.set_axon_ntff_profile_hook = lambda h: hook.__setitem__(0, h)
    mod.get_axon_ntff_profile_hook = lambda: hook[0]
    sys.modules["antenv.axon_hooks"] = mod
    try:
        import antenv
        antenv.axon_hooks = mod
    except ImportError:
        pass


def run(inputs, trace=False, **spmd_kwargs):
    """Run on the 8 NeuronCores; returns (out [256, 6144] f32, BassKernelResults)."""
    if trace:
        _install_ntff_hook()
    nc = _get_nc()
    in_maps = make_in_maps(**inputs)
    res = run_bass_kernel_spmd(nc, in_maps, core_ids=list(range(NCORES)),
                               trace=trace, **spmd_kwargs)
    out = np.concatenate([res.results[i]["out"] for i in range(NCORES)], axis=0)
    return out.astype(np.float32, copy=False), res


def kernel(x, x_hat, h_dec_prev, W_read, b_read):
    out, _ = run(dict(x=x, x_hat=x_hat, h_dec_prev=h_dec_prev,
                      W_read=W_read, b_read=b_read))
    return out


# revision 53
# speedup vs baseline: 1.0929x; 1.0327x over previous
"""DRAW-style read attention on Trainium2 — data-parallel over batch on 8 NeuronCores.

reference math (per batch element):
    params = h @ W.T + b                         [5]
    g_x = 64.5*(p0+1)-0.5 ; g_y likewise
    sigma2 = exp(p2) ; delta = (127/31)*exp(p3) ; gamma = exp(p4)
    mu_x[n] = g_x + (n-15.5)*delta ; mu_y likewise
    FX[n,h] = exp(-(h-mu_x[n])^2/(2 sigma2)) / (Z_n + 1e-8)    (Z_n = row sum)
    FY[m,w] likewise
    patch_i = FX @ img_i @ FY.T   for img in (x_c0..2, xhat_c0..2)
    out = gamma * flatten(patches)               [6144]

device layout per core (local batch B=32):
    params via 8 accumulated K=128 matmuls; tiny transforms on DVE/ACT
    expand per-b scalars to the (b,n)-partition layout with 0/1 selection matmuls
    filterbanks built in [bn, hw] layout (free-axis normalize), gamma folded into FY,
    then PE-transposed into FXT/FYT [hw, bn] bf16 for use as matmul rhs
    main loop over b: At[w,n] = img[h,w].T @ FXT_b ; patch[n,m] = At.T @ FYT_b
    (FX normalizer folded into FXT, FY normalizer*gamma folded into FYT, so patch
    in PSUM is final) -> ACT copy -> DMA to out rows
"""

import math

import numpy as np

import concourse.bass as bass  # noqa: F401  (import keeps bass registered)
import concourse.mybir as mybir
import concourse.tile as tile
from concourse import bacc
from concourse.bass_utils import run_bass_kernel_spmd
from concourse.masks import make_identity
from concourse.tile_rust import add_dep_helper

F32 = mybir.dt.float32
BF16 = mybir.dt.bfloat16

NCORES = 8
B = 32          # per-core batch shard
C = 3
IMG = 128
N = 32
DH = 1024
U = 2 * C       # images per batch element: x channels 0..2 then x_hat channels 0..2
NT = (B * N) // 128   # tiles over the flattened (b, n) axis
KC = DH // 128        # contraction chunks for the params matmul
DELTA_NORM = (IMG - 1.0) / (N - 1.0)
EPS = 1e-8


def build_nc(finalize=True):
    nc = bacc.Bacc("TRN2", target_bir_lowering=False, debug=False, num_devices=NCORES)
    AFT = mybir.ActivationFunctionType
    ALU = mybir.AluOpType

    x_d = nc.declare_dram_parameter("x", [B, C, IMG, IMG], F32, isOutput=False)
    xh_d = nc.declare_dram_parameter("xh", [B, C, IMG, IMG], F32, isOutput=False)
    # h and W arrive as bf16 hi/lo splits packed side by side: v = hi + lo with
    # |lo| ~ 2^-9 |v|, so hi@Whi + hi@Wlo + lo@Whi reproduces the f32 product
    # to ~1e-5 rel at bf16 matmul speed. Constants are packed into few tensors
    # so the startup isn't serialized by per-DMA queue overheads.
    hT2_d = nc.declare_dram_parameter("hT2", [DH, 2 * B], BF16, isOutput=False)
    wT2_d = nc.declare_dram_parameter("wT2", [DH, 10], BF16, isOutput=False)
    bc_d = nc.declare_dram_parameter("bc", [B, 11], F32, isOutput=False)
    E_d = nc.declare_dram_parameter("E", [NT, B, 128], BF16, isOutput=False)
    go_d = nc.declare_dram_parameter("go", [128, IMG + 1], F32, isOutput=False)
    out_d = nc.declare_dram_parameter("out", [B, U * N * N], F32, isOutput=True)

    with tile.TileContext(nc) as tc:
        with (
            tc.tile_pool(name="consts", bufs=1) as consts,
            tc.tile_pool(name="fb", bufs=4) as fb,
            tc.tile_pool(name="imgf_p", bufs=4) as imgf_p,
            tc.tile_pool(name="imgb_p", bufs=3) as imgb_p,
            tc.tile_pool(name="atb_p", bufs=3) as atb_p,
            tc.tile_pool(name="outs_p", bufs=3) as outs_p,
            tc.tile_pool(name="ps_pro", bufs=1, space="PSUM") as ps_pro,
            tc.tile_pool(name="ps_tr", bufs=2, space="PSUM") as ps_tr,
            tc.tile_pool(name="ps_at", bufs=2, space="PSUM") as ps_at,
            tc.tile_pool(name="ps_pt", bufs=2, space="PSUM") as ps_pt,
        ):
            # ---- constants / small inputs ----
            # hot consts (params deps) first on the SP queue, ahead of images
            hT2_sb = consts.tile([128, KC, 2 * B], BF16)
            nc.sync.dma_start(out=hT2_sb, in_=hT2_d[:].rearrange("(k p) b -> p k b", p=128))
            wT2_sb = consts.tile([128, KC, 10], BF16)
            nc.sync.dma_start(out=wT2_sb, in_=wT2_d[:].rearrange("(k p) j -> p k j", p=128))
            bc_sb = consts.tile([B, 11], F32)
            nc.sync.dma_start(out=bc_sb, in_=bc_d[:])
            bias_sb = bc_sb[:, 0:5]
            colsc_sb = bc_sb[:, 5:11]
            # cold consts (expansion/filterbank deps) ride the ACT queue
            E_sb = consts.tile([B, NT, 128], BF16)
            nc.scalar.dma_start(out=E_sb, in_=E_d[:].rearrange("t b p -> b t p"))
            go_sb = consts.tile([128, IMG + 1], F32)
            nc.scalar.dma_start(out=go_sb, in_=go_d[:])
            grid_sb = go_sb[:, 0:IMG]
            offs_sb = go_sb[:, IMG:IMG + 1]
            ident = consts.tile([128, 128], BF16)
            make_identity(nc, ident)
            zeros = consts.tile([128, 1], F32)
            nc.vector.memset(zeros, 0.0)
            # prime the ACT function table at t~0 so the 1.3us table load
            # doesn't sit on the params->filterbank critical path
            prime_t = consts.tile([1, 1], F32)
            nc.scalar.activation(prime_t, zeros[:1], AFT.Exp, scale=-1.0,
                                 bias=zeros[:1])

            # ---- quad image loads: x on the SP HWDGE queue, x_hat on the ACT
            # HWDGE queue; f32->bf16 casts on DVE (x) and ACT (x_hat). DMAs
            # are issued as early as possible; the casts are emitted late so
            # the static per-engine schedule never parks a DMA-gated cast in
            # front of critical-path compute.
            def emit_quad_dma(t):
                imgf = imgf_p.tile([128, 2, 4, C, IMG], F32, tag="imgf",
                                   name=f"imgf{t}")
                nc.sync.dma_start(out=imgf[:, 0],
                                  in_=x_d[4 * t:4 * t + 4].rearrange("b c h w -> h (b c) w"))
                nc.scalar.dma_start(out=imgf[:, 1],
                                    in_=xh_d[4 * t:4 * t + 4].rearrange("b c h w -> h (b c) w"))
                return imgf

            fb_anchor = {}

            def emit_quad_cast(t, imgf):
                imgb = imgb_p.tile([128, 2, 4, C, IMG], BF16, tag="imgb",
                                   name=f"imgb{t}")
                xc = nc.vector.tensor_copy(imgb[:, 0], imgf[:, 0])
                hc = nc.scalar.copy(imgb[:, 1], imgf[:, 1])
                # pin the DMA-gated casts behind this tile's filterbank ops in
                # the static schedule — the scheduler's DMA model is optimistic
                # and would otherwise park them in front of critical-path work
                if t in fb_anchor:
                    dve_a, act_a = fb_anchor[t]
                    add_dep_helper(xc.ins, dve_a.ins, sync=False,
                                   reason="cast after fbank (sched order)")
                    add_dep_helper(hc.ins, act_a.ins, sync=False,
                                   reason="cast after fbank (sched order)")
                return imgb

            quad_imgf = {0: emit_quad_dma(0), 1: emit_quad_dma(1)}
            quad_imgb = {}

            # ---- params = h @ W.T + b  -> [B, 5] ----
            ps_par = ps_pro.tile([B, 5], F32)
            terms = [(slice(0, B), slice(0, 5)), (slice(0, B), slice(5, 10)),
                     (slice(B, 2 * B), slice(0, 5))]
            for k in range(KC):
                for ti, (hsl, wsl) in enumerate(terms):
                    nc.tensor.matmul(ps_par, hT2_sb[:, k, hsl], wT2_sb[:, k, wsl],
                                     start=(k == 0 and ti == 0),
                                     stop=(k == KC - 1 and ti == len(terms) - 1))
            tp = consts.tile([B, 5], F32)
            nc.vector.tensor_add(tp, ps_par, bias_sb)

            # ---- transforms -> tp2 cols = [g_x, g_y, s=sqrt(1/(2*sigma2)), delta, gamma]
            # cols 2..4 share one exp: exp([-0.5*p2, p3, p4]) * [sqrt(.5), 127/31, 1]
            tp2 = consts.tile([B, 5], F32)
            half = (IMG + 1) / 2.0
            nc.vector.tensor_scalar(tp2[:, 0:2], tp[:, 0:2], half, half - 0.5,
                                    op0=ALU.mult, op1=ALU.add)
            t3 = consts.tile([B, 3], F32)
            nc.vector.tensor_mul(t3, tp[:, 2:5], colsc_sb[:, 0:3])
            e3 = consts.tile([B, 3], F32)
            nc.scalar.activation(e3, t3, AFT.Exp, bias=zeros[:B])
            nc.vector.tensor_mul(tp2[:, 2:5], e3, colsc_sb[:, 3:6])

            # device-side hi/lo split of tp2 so the expansion matmuls run bf16
            # exactly (E is 0/1): expanded value = tp2h + tp2l = tp2
            tp2h = consts.tile([B, 5], BF16)
            nc.vector.tensor_copy(tp2h, tp2)
            tp2hf = consts.tile([B, 5], F32)
            nc.vector.tensor_copy(tp2hf, tp2h)
            tp2l = consts.tile([B, 5], BF16)
            nc.vector.tensor_sub(tp2l, tp2, tp2hf)

            # ---- expand per-b scalars to (b,n) partitions: ep [128, NT, 5] ----
            ps_e = ps_pro.tile([128, NT, 5], F32)
            for t in range(NT):
                nc.tensor.matmul(ps_e[:, t, :], E_sb[:, t, :], tp2h,
                                 start=True, stop=False)
                nc.tensor.matmul(ps_e[:, t, :], E_sb[:, t, :], tp2l,
                                 start=False, stop=True)
            # transposing copy so each parameter plane ep[:, j, :] is contiguous
            ep = consts.tile([128, 5, NT], F32)
            nc.vector.tensor_copy(ep.rearrange("p j t -> p t j"), ps_e)

            mu_x = consts.tile([128, NT], F32)
            nc.vector.scalar_tensor_tensor(mu_x, ep[:, 3, :], offs_sb, ep[:, 0, :],
                                           op0=ALU.mult, op1=ALU.add)
            mu_y = consts.tile([128, NT], F32)
            nc.vector.scalar_tensor_tensor(mu_y, ep[:, 3, :], offs_sb, ep[:, 1, :],
                                           op0=ALU.mult, op1=ALU.add)
            # bias terms for the Square trick: -mu*s
            nsmu_x = consts.tile([128, NT], F32)
            nc.vector.scalar_tensor_tensor(nsmu_x, mu_x, -1.0, ep[:, 2, :],
                                           op0=ALU.mult, op1=ALU.mult)
            nsmu_y = consts.tile([128, NT], F32)
            nc.vector.scalar_tensor_tensor(nsmu_y, mu_y, -1.0, ep[:, 2, :],
                                           op0=ALU.mult, op1=ALU.mult)

            # both filterbanks bf16 (matmuls run bf16); gamma folded into FY
            FXT = consts.tile([128, B * N], BF16)
            FYT = consts.tile([128, B * N], BF16)

            def fbank2(t):
                # sq = (s*grid - s*mu)^2 = (grid-mu)^2/(2 sigma2), X and Y
                # halves share one exp / reduce / reciprocal pass
                sq = fb.tile([128, 2, IMG], F32, tag="sq")
                nc.scalar.activation(sq[:, 0, :], grid_sb, AFT.Square,
                                     scale=ep[:, 2, t:t + 1], bias=nsmu_x[:, t:t + 1])
                nc.scalar.activation(sq[:, 1, :], grid_sb, AFT.Square,
                                     scale=ep[:, 2, t:t + 1], bias=nsmu_y[:, t:t + 1])
                e_un = fb.tile([128, 2, IMG], F32, tag="e_un")
                last_exp = nc.scalar.activation(e_un, sq, AFT.Exp, scale=-1.0,
                                                bias=zeros)
                Z2 = fb.tile([128, 2], F32, tag="Z2")
                nc.vector.tensor_reduce(Z2, e_un, axis=mybir.AxisListType.X,
                                        op=ALU.add)
                nc.vector.tensor_scalar_add(Z2, Z2, EPS)
                invZ2 = fb.tile([128, 2], F32, tag="invZ2")
                nc.vector.reciprocal(invZ2, Z2)
                nc.vector.tensor_mul(invZ2[:, 1:2], invZ2[:, 1:2], ep[:, 4, t:t + 1])
                last_dve = None
                for j, FT in ((0, FXT), (1, FYT)):
                    Fn = fb.tile([128, IMG], BF16, tag="Fn")
                    nc.vector.tensor_scalar_mul(Fn, e_un[:, j, :], invZ2[:, j:j + 1])
                    ps_t = ps_tr.tile([128, 128], BF16, tag="ps_t")
                    nc.tensor.transpose(ps_t, Fn, ident)
                    last_dve = nc.vector.tensor_copy(FT[:, t * 128:(t + 1) * 128], ps_t)
                fb_anchor[t] = (last_dve, last_exp)

            # ---- main loop: pairs of batch elements, interleaved with the
            # filterbank tiles they depend on; mm2 pipelined one pair behind ----
            # mm2 is column-tiled: unit u lands on PSUM partitions 32*(u%4) at
            # free slot u//4, so the epilogue copy runs at full 128-partition
            # width; the output view flattens (u%4, n) back into DRAM columns
            out_v = (out_d[:]
                     .rearrange("(P b2) (i c n m) -> P (b2 i c) n m",
                                b2=2, i=2, c=C, n=N)
                     .rearrange("P (s j) n m -> P j n s m", s=3))

            def mm1(P, imgb, pp):
                ps_a = ps_at.tile([128, 2, U, N], F32)
                for b2 in range(2):
                    b = 2 * P + b2
                    for i in range(2):
                        for c in range(C):
                            nc.tensor.matmul(ps_a[:, b2, i * C + c, :],
                                             imgb[:, i, 2 * pp + b2, c, :],
                                             FXT[:, b * N:(b + 1) * N],
                                             start=True, stop=True)
                atb = atb_p.tile([128, 2, U, N], BF16, tag="atb")
                nc.vector.tensor_copy(atb, ps_a)
                return atb

            def mm2_store(P, atb):
                ps_p = ps_pt.tile([128, 3, N], F32)
                for b2 in range(2):
                    b = 2 * P + b2
                    for u in range(U):
                        up = b2 * U + u
                        j, slot = up % 4, up // 4
                        nc.tensor.matmul(ps_p[32 * j:32 * (j + 1), slot, :],
                                         atb[:, b2, u, :],
                                         FYT[:, b * N:(b + 1) * N],
                                         start=True, stop=True,
                                         tile_position=(0, 32 * j))
                outs = outs_p.tile([128, 3, N], F32, tag="outs")
                nc.vector.tensor_copy(outs, ps_p)
                # out DMA rides the gpsimd SWDGE queue (own queue, idle engine)
                nc.gpsimd.dma_start(out=out_v[P], in_=outs)

            prev = None
            fbank2(0)
            fbank2(1)
            quad_imgb[0] = emit_quad_cast(0, quad_imgf.pop(0))
            for t in range(NT):
                # DMAs two quads ahead, filterbanks two tiles ahead, casts one
                # quad ahead: every dependency lands before its consumer
                if t + 2 < NT:
                    quad_imgf[t + 2] = emit_quad_dma(t + 2)
                    fbank2(t + 2)
                if t + 1 < NT:
                    quad_imgb[t + 1] = emit_quad_cast(t + 1, quad_imgf.pop(t + 1))
                imgb = quad_imgb.pop(t)
                for pp in range(2):
                    P = 2 * t + pp
                    atb = mm1(P, imgb, pp)
                    if prev is not None:
                        mm2_store(*prev)
                    prev = (P, atb)
            mm2_store(*prev)

    if finalize:
        nc.finalize()
    return nc


_CACHE = {}


def _get_nc():
    if "nc" not in _CACHE:
        _CACHE["nc"] = build_nc()
    return _CACHE["nc"]


def host_constants():
    import ml_dtypes
    E = np.zeros((NT, B, 128), ml_dtypes.bfloat16)
    for t in range(NT):
        for p in range(128):
            E[t, (t * 128 + p) // N, p] = 1.0
    offs = (np.arange(128) % N - (N / 2.0 - 0.5)).astype(np.float32).reshape(128, 1)
    grid = np.broadcast_to(np.arange(IMG, dtype=np.float32), (128, IMG))
    go = np.ascontiguousarray(np.concatenate([grid, offs], axis=1))
    colsc = np.broadcast_to(
        np.array([-0.5, 1.0, 1.0, math.sqrt(0.5), DELTA_NORM, 1.0], np.float32),
        (B, 6))
    return E, go, colsc


def make_in_maps(x, x_hat, h_dec_prev, W_read, b_read):
    x = np.asarray(x, np.float32)
    x_hat = np.asarray(x_hat, np.float32)
    h = np.asarray(h_dec_prev, np.float32)
    E, go, colsc = host_constants()
    import ml_dtypes
    bf16 = ml_dtypes.bfloat16

    def hilo2(a):
        hi = a.astype(bf16)
        lo = (a - hi.astype(np.float32)).astype(bf16)
        return np.ascontiguousarray(np.concatenate([hi, lo], axis=1))

    wT2 = hilo2(np.asarray(W_read, np.float32).T)
    bias = np.broadcast_to(np.asarray(b_read, np.float32), (B, 5))
    bc = np.ascontiguousarray(np.concatenate([bias, colsc], axis=1))
    in_maps = []
    for i in range(NCORES):
        sl = slice(i * B, (i + 1) * B)
        in_maps.append({
            "x": np.ascontiguousarray(x[sl]),
            "xh": np.ascontiguousarray(x_hat[sl]),
            "hT2": hilo2(np.ascontiguousarray(h[sl].T)),
            "wT2": wT2,
            "bc": bc,
            "E": E,
            "go": go,
        })
    return in_maps


def _install_ntff_hook():
    """The container's antenv package lacks axon_hooks; provide it so
    run_bass_kernel_spmd(trace=True) can capture an NTFF profile."""
    import sys
    import types
    if "antenv.axon_hooks" in sys.modules:
        return
    try:
        from trn_agent_boot.trn_boot import _ntff_profile_via_ctypes
    except ImportError:
        return
    mod = types.ModuleType("antenv.axon_hooks")
    hook = [_ntff_profile_via_ctypes("/opt/axon/libaxon_pjrt.so")]
    mod.set_axon_ntff_profile_hook = lambda h: hook.__setitem__(0, h)
    mod.get_axon_ntff_profile_hook = lambda: hook[0]
    sys.modules["antenv.axon_hooks"] = mod
    try:
        import antenv
        antenv.axon_hooks = mod
    except ImportError:
        pass


def run(inputs, trace=False, **spmd_kwargs):
    """Run on the 8 NeuronCores; returns (out [256, 6144] f32, BassKernelResults)."""
    if trace:
        _install_ntff_hook()
    nc = _get_nc()
    in_maps = make_in_maps(**inputs)
    res = run_bass_kernel_spmd(nc, in_maps, core_ids=list(range(NCORES)),
                               trace=trace, **spmd_kwargs)
    out = np.concatenate([res.results[i]["out"] for i in range(NCORES)], axis=0)
    return out.astype(np.float32, copy=False), res


def kernel(x, x_hat, h_dec_prev, W_read, b_read):
    out, _ = run(dict(x=x, x_hat=x_hat, h_dec_prev=h_dec_prev,
                      W_read=W_read, b_read=b_read))
    return out


# revision 55
# speedup vs baseline: 1.2033x; 1.1011x over previous
"""DRAW-style read attention on Trainium2 — data-parallel over batch on 8 NeuronCores.

reference math (per batch element):
    params = h @ W.T + b                         [5]
    g_x = 64.5*(p0+1)-0.5 ; g_y likewise
    sigma2 = exp(p2) ; delta = (127/31)*exp(p3) ; gamma = exp(p4)
    mu_x[n] = g_x + (n-15.5)*delta ; mu_y likewise
    FX[n,h] = exp(-(h-mu_x[n])^2/(2 sigma2)) / (Z_n + 1e-8)    (Z_n = row sum)
    FY[m,w] likewise
    patch_i = FX @ img_i @ FY.T   for img in (x_c0..2, xhat_c0..2)
    out = gamma * flatten(patches)               [6144]

device layout per core (local batch B=32):
    params via 8 accumulated K=128 matmuls; tiny transforms on DVE/ACT
    expand per-b scalars to the (b,n)-partition layout with 0/1 selection matmuls
    filterbanks built in [bn, hw] layout (free-axis normalize), gamma folded into FY,
    then PE-transposed into FXT/FYT [hw, bn] bf16 for use as matmul rhs
    main loop over b: At[w,n] = img[h,w].T @ FXT_b ; patch[n,m] = At.T @ FYT_b
    (FX normalizer folded into FXT, FY normalizer*gamma folded into FYT, so patch
    in PSUM is final) -> ACT copy -> DMA to out rows
"""

import math

import numpy as np

import concourse.bass as bass  # noqa: F401  (import keeps bass registered)
import concourse.mybir as mybir
import concourse.tile as tile
from concourse import bacc
from concourse.bass_utils import run_bass_kernel_spmd
from concourse.masks import make_identity
from concourse.tile_rust import add_dep_helper

F32 = mybir.dt.float32
BF16 = mybir.dt.bfloat16

NCORES = 8
B = 32          # per-core batch shard
C = 3
IMG = 128
N = 32
DH = 1024
U = 2 * C       # images per batch element: x channels 0..2 then x_hat channels 0..2
NT = (B * N) // 128   # tiles over the flattened (b, n) axis
KC = DH // 128        # contraction chunks for the params matmul
DELTA_NORM = (IMG - 1.0) / (N - 1.0)
EPS = 1e-8


def build_nc(finalize=True):
    nc = bacc.Bacc("TRN2", target_bir_lowering=False, debug=False, num_devices=NCORES)
    AFT = mybir.ActivationFunctionType
    ALU = mybir.AluOpType

    x_d = nc.declare_dram_parameter("x", [B, C, IMG, IMG], F32, isOutput=False)
    xh_d = nc.declare_dram_parameter("xh", [B, C, IMG, IMG], F32, isOutput=False)
    # h and W arrive as bf16 hi/lo splits packed side by side: v = hi + lo with
    # |lo| ~ 2^-9 |v|, so hi@Whi + hi@Wlo + lo@Whi reproduces the f32 product
    # to ~1e-5 rel at bf16 matmul speed. Constants are packed into few tensors
    # so the startup isn't serialized by per-DMA queue overheads.
    hT2_d = nc.declare_dram_parameter("hT2", [DH, 2 * B], BF16, isOutput=False)
    wT2_d = nc.declare_dram_parameter("wT2", [DH, 10], BF16, isOutput=False)
    bc_d = nc.declare_dram_parameter("bc", [B, 11], F32, isOutput=False)
    E_d = nc.declare_dram_parameter("E", [NT, B, 128], BF16, isOutput=False)
    go_d = nc.declare_dram_parameter("go", [128, IMG + 1], F32, isOutput=False)
    out_d = nc.declare_dram_parameter("out", [B, U * N * N], F32, isOutput=True)

    with tile.TileContext(nc) as tc:
        with (
            tc.tile_pool(name="consts", bufs=1) as consts,
            tc.tile_pool(name="fb", bufs=4) as fb,
            tc.tile_pool(name="imgf_p", bufs=4) as imgf_p,
            tc.tile_pool(name="imgb_p", bufs=3) as imgb_p,
            tc.tile_pool(name="atb_p", bufs=3) as atb_p,
            tc.tile_pool(name="outs_p", bufs=3) as outs_p,
            tc.tile_pool(name="ps_pro", bufs=1, space="PSUM") as ps_pro,
            tc.tile_pool(name="ps_tr", bufs=2, space="PSUM") as ps_tr,
            tc.tile_pool(name="ps_at", bufs=2, space="PSUM") as ps_at,
            tc.tile_pool(name="ps_pt", bufs=2, space="PSUM") as ps_pt,
        ):
            # ---- constants / small inputs ----
            # hot consts (params deps) first on the SP queue, ahead of images
            hT2_sb = consts.tile([128, KC, 2 * B], BF16)
            nc.sync.dma_start(out=hT2_sb, in_=hT2_d[:].rearrange("(k p) b -> p k b", p=128))
            wT2_sb = consts.tile([128, KC, 10], BF16)
            nc.sync.dma_start(out=wT2_sb, in_=wT2_d[:].rearrange("(k p) j -> p k j", p=128))
            bc_sb = consts.tile([B, 11], F32)
            nc.sync.dma_start(out=bc_sb, in_=bc_d[:])
            bias_sb = bc_sb[:, 0:5]
            colsc_sb = bc_sb[:, 5:11]
            # cold consts (expansion/filterbank deps) ride the ACT queue
            E_sb = consts.tile([B, NT, 128], BF16)
            nc.scalar.dma_start(out=E_sb, in_=E_d[:].rearrange("t b p -> b t p"))
            go_sb = consts.tile([128, IMG + 1], F32)
            nc.scalar.dma_start(out=go_sb, in_=go_d[:])
            grid_sb = go_sb[:, 0:IMG]
            offs_sb = go_sb[:, IMG:IMG + 1]
            ident = consts.tile([128, 128], BF16)
            make_identity(nc, ident)
            zeros = consts.tile([128, 1], F32)
            nc.vector.memset(zeros, 0.0)
            # prime the ACT function table at t~0 so the 1.3us table load
            # doesn't sit on the params->filterbank critical path
            prime_t = consts.tile([1, 1], F32)
            nc.scalar.activation(prime_t, zeros[:1], AFT.Exp, scale=-1.0,
                                 bias=zeros[:1])

            # ---- quad image loads: x on the SP HWDGE queue, x_hat on the ACT
            # HWDGE queue; f32->bf16 casts on DVE (x) and ACT (x_hat). DMAs
            # are issued as early as possible; the casts are emitted late so
            # the static per-engine schedule never parks a DMA-gated cast in
            # front of critical-path compute.
            def emit_quad_dma(t):
                imgf = imgf_p.tile([128, 2, 4, C, IMG], F32, tag="imgf",
                                   name=f"imgf{t}")
                nc.sync.dma_start(out=imgf[:, 0],
                                  in_=x_d[4 * t:4 * t + 4].rearrange("b c h w -> h (b c) w"))
                nc.scalar.dma_start(out=imgf[:, 1],
                                    in_=xh_d[4 * t:4 * t + 4].rearrange("b c h w -> h (b c) w"))
                return imgf

            fb_anchor = {}

            def emit_quad_cast(t, imgf):
                imgb = imgb_p.tile([128, 2, 4, C, IMG], BF16, tag="imgb",
                                   name=f"imgb{t}")
                # gpsimd (otherwise idle) absorbs one b-slice of the x cast
                nc.gpsimd.tensor_copy(imgb[:, 0, 3], imgf[:, 0, 3])
                xc = nc.vector.tensor_copy(imgb[:, 0, 0:3], imgf[:, 0, 0:3])
                hc = nc.scalar.copy(imgb[:, 1], imgf[:, 1])
                # pin the DMA-gated casts behind this tile's filterbank ops in
                # the static schedule — the scheduler's DMA model is optimistic
                # and would otherwise park them in front of critical-path work
                if t in fb_anchor:
                    dve_a, act_a = fb_anchor[t]
                    add_dep_helper(xc.ins, dve_a.ins, sync=False,
                                   reason="cast after fbank (sched order)")
                    add_dep_helper(hc.ins, act_a.ins, sync=False,
                                   reason="cast after fbank (sched order)")
                return imgb

            quad_imgf = {0: emit_quad_dma(0), 1: emit_quad_dma(1)}
            quad_imgb = {}

            # ---- params = h @ W.T + b  -> [B, 5] ----
            ps_par = ps_pro.tile([B, 5], F32)
            terms = [(slice(0, B), slice(0, 5)), (slice(0, B), slice(5, 10)),
                     (slice(B, 2 * B), slice(0, 5))]
            for k in range(KC):
                for ti, (hsl, wsl) in enumerate(terms):
                    nc.tensor.matmul(ps_par, hT2_sb[:, k, hsl], wT2_sb[:, k, wsl],
                                     start=(k == 0 and ti == 0),
                                     stop=(k == KC - 1 and ti == len(terms) - 1))
            tp = consts.tile([B, 5], F32)
            nc.vector.tensor_add(tp, ps_par, bias_sb)

            # ---- transforms -> tp2 cols = [g_x, g_y, s=sqrt(1/(2*sigma2)), delta, gamma]
            # cols 2..4 share one exp: exp([-0.5*p2, p3, p4]) * [sqrt(.5), 127/31, 1]
            tp2 = consts.tile([B, 5], F32)
            half = (IMG + 1) / 2.0
            nc.vector.tensor_scalar(tp2[:, 0:2], tp[:, 0:2], half, half - 0.5,
                                    op0=ALU.mult, op1=ALU.add)
            t3 = consts.tile([B, 3], F32)
            nc.vector.tensor_mul(t3, tp[:, 2:5], colsc_sb[:, 0:3])
            e3 = consts.tile([B, 3], F32)
            nc.scalar.activation(e3, t3, AFT.Exp, bias=zeros[:B])
            nc.vector.tensor_mul(tp2[:, 2:5], e3, colsc_sb[:, 3:6])

            # device-side hi/lo split of tp2 so the expansion matmuls run bf16
            # exactly (E is 0/1): expanded value = tp2h + tp2l = tp2
            tp2h = consts.tile([B, 5], BF16)
            nc.vector.tensor_copy(tp2h, tp2)
            tp2hf = consts.tile([B, 5], F32)
            nc.vector.tensor_copy(tp2hf, tp2h)
            tp2l = consts.tile([B, 5], BF16)
            nc.vector.tensor_sub(tp2l, tp2, tp2hf)

            # ---- expand per-b scalars to (b,n) partitions: ep [128, NT, 5] ----
            ps_e = ps_pro.tile([128, NT, 5], F32)
            for t in range(NT):
                nc.tensor.matmul(ps_e[:, t, :], E_sb[:, t, :], tp2h,
                                 start=True, stop=False)
                nc.tensor.matmul(ps_e[:, t, :], E_sb[:, t, :], tp2l,
                                 start=False, stop=True)
            # transposing copy so each parameter plane ep[:, j, :] is contiguous
            ep = consts.tile([128, 5, NT], F32)
            nc.vector.tensor_copy(ep.rearrange("p j t -> p t j"), ps_e)

            mu_x = consts.tile([128, NT], F32)
            nc.vector.scalar_tensor_tensor(mu_x, ep[:, 3, :], offs_sb, ep[:, 0, :],
                                           op0=ALU.mult, op1=ALU.add)
            mu_y = consts.tile([128, NT], F32)
            nc.vector.scalar_tensor_tensor(mu_y, ep[:, 3, :], offs_sb, ep[:, 1, :],
                                           op0=ALU.mult, op1=ALU.add)
            # bias terms for the Square trick: -mu*s
            nsmu_x = consts.tile([128, NT], F32)
            nc.vector.scalar_tensor_tensor(nsmu_x, mu_x, -1.0, ep[:, 2, :],
                                           op0=ALU.mult, op1=ALU.mult)
            nsmu_y = consts.tile([128, NT], F32)
            nc.vector.scalar_tensor_tensor(nsmu_y, mu_y, -1.0, ep[:, 2, :],
                                           op0=ALU.mult, op1=ALU.mult)

            # both filterbanks bf16 (matmuls run bf16); gamma folded into FY
            FXT = consts.tile([128, B * N], BF16)
            FYT = consts.tile([128, B * N], BF16)

            def fbank2(t):
                # sq = (s*grid - s*mu)^2 = (grid-mu)^2/(2 sigma2), X and Y
                # halves share one exp / reduce / reciprocal pass
                sq = fb.tile([128, 2, IMG], F32, tag="sq")
                nc.scalar.activation(sq[:, 0, :], grid_sb, AFT.Square,
                                     scale=ep[:, 2, t:t + 1], bias=nsmu_x[:, t:t + 1])
                nc.scalar.activation(sq[:, 1, :], grid_sb, AFT.Square,
                                     scale=ep[:, 2, t:t + 1], bias=nsmu_y[:, t:t + 1])
                e_un = fb.tile([128, 2, IMG], F32, tag="e_un")
                last_exp = nc.scalar.activation(e_un, sq, AFT.Exp, scale=-1.0,
                                                bias=zeros)
                Z2 = fb.tile([128, 2], F32, tag="Z2")
                nc.vector.tensor_reduce(Z2, e_un, axis=mybir.AxisListType.X,
                                        op=ALU.add)
                nc.vector.tensor_scalar_add(Z2, Z2, EPS)
                invZ2 = fb.tile([128, 2], F32, tag="invZ2")
                nc.vector.reciprocal(invZ2, Z2)
                nc.vector.tensor_mul(invZ2[:, 1:2], invZ2[:, 1:2], ep[:, 4, t:t + 1])
                last_dve = None
                for j, FT in ((0, FXT), (1, FYT)):
                    Fn = fb.tile([128, IMG], BF16, tag="Fn")
                    nc.vector.tensor_scalar_mul(Fn, e_un[:, j, :], invZ2[:, j:j + 1])
                    ps_t = ps_tr.tile([128, 128], BF16, tag="ps_t")
                    nc.tensor.transpose(ps_t, Fn, ident)
                    last_dve = nc.vector.tensor_copy(FT[:, t * 128:(t + 1) * 128], ps_t)
                fb_anchor[t] = (last_dve, last_exp)

            # ---- main loop: pairs of batch elements, interleaved with the
            # filterbank tiles they depend on; mm2 pipelined one pair behind ----
            # mm2 is column-tiled: unit u lands on PSUM partitions 32*(u%4) at
            # free slot u//4, so the epilogue copy runs at full 128-partition
            # width; the output view flattens (u%4, n) back into DRAM columns
            out_v = (out_d[:]
                     .rearrange("(P b2) (i c n m) -> P (b2 i c) n m",
                                b2=2, i=2, c=C, n=N)
                     .rearrange("P (s j) n m -> P j n s m", s=3))

            def mm1(P, imgb, pp):
                ps_a = ps_at.tile([128, 2, U, N], F32)
                for b2 in range(2):
                    b = 2 * P + b2
                    for i in range(2):
                        for c in range(C):
                            nc.tensor.matmul(ps_a[:, b2, i * C + c, :],
                                             imgb[:, i, 2 * pp + b2, c, :],
                                             FXT[:, b * N:(b + 1) * N],
                                             start=True, stop=True)
                atb = atb_p.tile([128, 2, U, N], BF16, tag="atb")
                nc.vector.tensor_copy(atb, ps_a)
                return atb

            def mm2_store(P, atb):
                ps_p = ps_pt.tile([128, 3, N], F32)
                for b2 in range(2):
                    b = 2 * P + b2
                    for u in range(U):
                        up = b2 * U + u
                        j, slot = up % 4, up // 4
                        nc.tensor.matmul(ps_p[32 * j:32 * (j + 1), slot, :],
                                         atb[:, b2, u, :],
                                         FYT[:, b * N:(b + 1) * N],
                                         start=True, stop=True,
                                         tile_position=(0, 32 * j))
                outs = outs_p.tile([128, 3, N], F32, tag="outs")
                nc.vector.tensor_copy(outs, ps_p)
                # out DMAs interleave on the two HW queues (SWDGE transfers
                # measured ~8us of straggle after descriptor gen)
                eng = nc.sync if P % 2 == 0 else nc.scalar
                eng.dma_start(out=out_v[P], in_=outs)

            prev = None
            fbank2(0)
            fbank2(1)
            quad_imgb[0] = emit_quad_cast(0, quad_imgf.pop(0))
            for t in range(NT):
                # DMAs two quads ahead, filterbanks two tiles ahead, casts one
                # quad ahead: every dependency lands before its consumer
                if t + 2 < NT:
                    quad_imgf[t + 2] = emit_quad_dma(t + 2)
                    fbank2(t + 2)
                if t + 1 < NT:
                    quad_imgb[t + 1] = emit_quad_cast(t + 1, quad_imgf.pop(t + 1))
                imgb = quad_imgb.pop(t)
                for pp in range(2):
                    P = 2 * t + pp
                    atb = mm1(P, imgb, pp)
                    if prev is not None:
                        mm2_store(*prev)
                    prev = (P, atb)
            mm2_store(*prev)

    if finalize:
        nc.finalize()
    return nc


_CACHE = {}


def _get_nc():
    if "nc" not in _CACHE:
        _CACHE["nc"] = build_nc()
    return _CACHE["nc"]


def host_constants():
    import ml_dtypes
    E = np.zeros((NT, B, 128), ml_dtypes.bfloat16)
    for t in range(NT):
        for p in range(128):
            E[t, (t * 128 + p) // N, p] = 1.0
    offs = (np.arange(128) % N - (N / 2.0 - 0.5)).astype(np.float32).reshape(128, 1)
    grid = np.broadcast_to(np.arange(IMG, dtype=np.float32), (128, IMG))
    go = np.ascontiguousarray(np.concatenate([grid, offs], axis=1))
    colsc = np.broadcast_to(
        np.array([-0.5, 1.0, 1.0, math.sqrt(0.5), DELTA_NORM, 1.0], np.float32),
        (B, 6))
    return E, go, colsc


def make_in_maps(x, x_hat, h_dec_prev, W_read, b_read):
    x = np.asarray(x, np.float32)
    x_hat = np.asarray(x_hat, np.float32)
    h = np.asarray(h_dec_prev, np.float32)
    E, go, colsc = host_constants()
    import ml_dtypes
    bf16 = ml_dtypes.bfloat16

    def hilo2(a):
        hi = a.astype(bf16)
        lo = (a - hi.astype(np.float32)).astype(bf16)
        return np.ascontiguousarray(np.concatenate([hi, lo], axis=1))

    wT2 = hilo2(np.asarray(W_read, np.float32).T)
    bias = np.broadcast_to(np.asarray(b_read, np.float32), (B, 5))
    bc = np.ascontiguousarray(np.concatenate([bias, colsc], axis=1))
    in_maps = []
    for i in range(NCORES):
        sl = slice(i * B, (i + 1) * B)
        in_maps.append({
            "x": np.ascontiguousarray(x[sl]),
            "xh": np.ascontiguousarray(x_hat[sl]),
            "hT2": hilo2(np.ascontiguousarray(h[sl].T)),
            "wT2": wT2,
            "bc": bc,
            "E": E,
            "go": go,
        })
    return in_maps


def _install_ntff_hook():
    """The container's antenv package lacks axon_hooks; provide it so
    run_bass_kernel_spmd(trace=True) can capture an NTFF profile."""
    import sys
    import types
    if "antenv.axon_hooks" in sys.modules:
        return
    try:
        from trn_agent_boot.trn_boot import _ntff_profile_via_ctypes
    except ImportError:
        return
    mod = types.ModuleType("antenv.axon_hooks")
    hook = [_ntff_profile_via_ctypes("/opt/axon/libaxon_pjrt.so")]
    mod.set_axon_ntff_profile_hook = lambda h: hook.__setitem__(0, h)
    mod.get_axon_ntff_profile_hook = lambda: hook[0]
    sys.modules["antenv.axon_hooks"] = mod
    try:
        import antenv
        antenv.axon_hooks = mod
    except ImportError:
        pass


def run(inputs, trace=False, **spmd_kwargs):
    """Run on the 8 NeuronCores; returns (out [256, 6144] f32, BassKernelResults)."""
    if trace:
        _install_ntff_hook()
    nc = _get_nc()
    in_maps = make_in_maps(**inputs)
    res = run_bass_kernel_spmd(nc, in_maps, core_ids=list(range(NCORES)),
                               trace=trace, **spmd_kwargs)
    out = np.concatenate([res.results[i]["out"] for i in range(NCORES)], axis=0)
    return out.astype(np.float32, copy=False), res


def kernel(x, x_hat, h_dec_prev, W_read, b_read):
    out, _ = run(dict(x=x, x_hat=x_hat, h_dec_prev=h_dec_prev,
                      W_read=W_read, b_read=b_read))
    return out
